# revision 6
# baseline (speedup 1.0000x reference)
"""Trainium2 Bass kernel for ChameleonVQVAEEncoderAttnBlock (fp8 DoubleRow).

Reference computation (per batch b of 16, C=512 channels, N=32*32=1024):
    h  = GroupNorm32(x) * gamma + beta
    q, k, v = wq@h+bq, wk@h+bk, wv@h+bv      (1x1 convs == channel matmuls)
    S[i,j] = sum_c q[c,i] k[c,j] / sqrt(C);  A = softmax_j(S)
    o[c,i] = sum_j v[c,j] A[i,j];            y = wo@o + bo + x

Sharding: pure data parallel, batch 16 -> 2 batches on each of 8 cores.

Per-core strategy (all four big matmuls in fp8-e4m3 with the PE's
DoubleRow perf mode: 256-deep contraction per pass, 0.5 cyc/out-col):
  - Q/K fused host-side:  g = (wq^T wk)^T h, so S^T = g^T h.
  - V/O fused host-side:  vt = h^T (wv^T wo^T), so the attention-weighted
    sum over v directly produces the output projection.
  - S is computed transposed (j on partitions) so softmax's sum over j is
    a ones-stationary matmul and E feeds the AV matmul untransposed.
  - exp bias -2 keeps E in e4m3 range (softmax shift-invariance).
  - fp8 quant scales (x16 per operand) fold into gamma/beta (host), the
    PSUM-drain activation scales, and the 1/16 in the rdb ones-row.
  - Softmax denominator: DoubleRow ones matmul -> reciprocal -> ones-row
    f32r matmul broadcast -> [128, N] rdb in SBUF.
  - Drains: ACT does g/vt/exp/rdb, DVE does h-affine + AV*rdb, Pool
    (gpsimd) does the residual add.  GroupNorm stats are batched [128,4]
    ops + one block-diag 1/16 matmul for group aggregation+broadcast.
"""
import numpy as np
import ml_dtypes

import concourse.bacc as bacc
import concourse.mybir as mybir
import concourse.tile as tile
from concourse import bass_utils

F32 = mybir.dt.float32
F32R = mybir.dt.float32r
E4 = mybir.dt.float8e4
AF = mybir.ActivationFunctionType
ALU = mybir.AluOpType
DR = mybir.MatmulPerfMode.DoubleRow

B, C, HH, WW = 16, 512, 32, 32
N = HH * WW          # 1024 spatial positions
NCORES = 8
NB = B // NCORES     # batches per core
CCH = C // 128       # 4 channel chunks
SCH = N // 128       # 8 spatial chunks
NIH = N // 512       # 2 free-dim halves
GROUPS = 32
GPC = C // GROUPS    # 16 channels per group
EPS = 1e-6
SCALE = float(C) ** -0.5
SH = 16.0            # fp8 scale for h
SG = 16.0            # fp8 scale for g
SV = 16.0            # fp8 scale for vt
SW = 128.0           # fp8 scale for wm/wp
EBIAS = -2.0         # exp bias (softmax shift-invariant)


def _build_program(reps: int = 1, loop_n: int = 1, x_outside: bool = False):
    nc = bacc.Bacc("TRN2", target_bir_lowering=False, debug=False)

    x_d = nc.dram_tensor("x", [NB, C, N], F32, kind="ExternalInput").ap()
    wm_d = nc.dram_tensor("wm8", [CCH, 128, C], E4, kind="ExternalInput").ap()
    wp_d = nc.dram_tensor("wp8", [CCH, 128, C], E4, kind="ExternalInput").ap()
    full8_d = nc.dram_tensor("full8", [128, 2, 128], E4, kind="ExternalInput").ap()
    gam_d = nc.dram_tensor("gamma16", [C], F32, kind="ExternalInput").ap()
    bet_d = nc.dram_tensor("beta16", [C], F32, kind="ExternalInput").ap()
    wsel_d = nc.dram_tensor("wsel", [128, 128], F32, kind="ExternalInput").ap()
    y_d = nc.dram_tensor("y", [NB, C, N], F32, kind="ExternalOutput").ap()

    with tile.TileContext(nc) as tc:
        with (
            tc.tile_pool(name="const", bufs=1) as cp,
            tc.tile_pool(name="data", bufs=1) as dp,
            tc.tile_pool(name="psum", bufs=1, space="PSUM") as pp,
        ):
            # ---- small constants ----------------------------------------
            gam4 = cp.tile([128, CCH], F32)
            bet4 = cp.tile([128, CCH], F32)
            for cc in range(CCH):
                nc.sync.dma_start(out=gam4[:, cc:cc + 1],
                                  in_=gam_d[cc * 128:(cc + 1) * 128])
                nc.sync.dma_start(out=bet4[:, cc:cc + 1],
                                  in_=bet_d[cc * 128:(cc + 1) * 128])
            wsel = cp.tile([128, 128], F32)
            nc.sync.dma_start(out=wsel, in_=wsel_d)
            full8 = cp.tile([128, 2, 128], E4)
            nc.sync.dma_start(out=full8, in_=full8_d)
            eps_t = cp.tile([128, 1], F32)
            nc.vector.memset(eps_t, EPS)
            ebias_t = cp.tile([128, 1], F32)
            nc.vector.memset(ebias_t, EBIAS)

            wm_t = cp.tile([128, CCH, C], E4)
            wp_t = cp.tile([128, CCH, C], E4)

            def load_w():
                for cc in range(CCH):
                    nc.sync.dma_start(out=wm_t[:, cc, :], in_=wm_d[cc])
                for cc in range(CCH):
                    nc.sync.dma_start(out=wp_t[:, cc, :], in_=wp_d[cc])

            preloaded_x = [None]

            def emit_rep(r, load_weights=True):
                p = f"r{r}_"

                if preloaded_x[0] is not None:
                    x_t = preloaded_x[0]
                else:
                    x_t = [[
                        dp.tile([128, N], F32, name=f"{p}x{cc}_b{b}",
                                tag=f"x{cc}", bufs=2)
                        for cc in range(CCH)] for b in range(NB)]
                    for b in range(NB):
                        for cc in range(CCH):
                            for ih in range(NIH):
                                nc.sync.dma_start(
                                    out=x_t[b][cc][:, ih * 512:(ih + 1) * 512],
                                    in_=x_d[b, cc * 128:(cc + 1) * 128,
                                            ih * 512:(ih + 1) * 512],
                                )
                if r == 0 and load_weights:
                    load_w()

                h8 = [dp.tile([128, CCH, N], E4, name=f"{p}h_b{b}", tag="h8",
                              bufs=2) for b in range(NB)]
                g8 = [dp.tile([128, CCH, N], E4, name=f"{p}g_b{b}", tag="g8",
                              bufs=2) for b in range(NB)]
                e8 = [dp.tile([128, SCH, N], E4, name=f"{p}e_b{b}", tag="e8",
                              bufs=2) for b in range(NB)]
                vt8 = [dp.tile([128, SCH, C], E4, name=f"{p}vt_b{b}", tag="vt8",
                               bufs=2) for b in range(NB)]
                rdb = [dp.tile([128, N], F32, name=f"{p}rdb_b{b}", tag="rdb",
                               bufs=2) for b in range(NB)]

                # ---- groupnorm + h8 (fp8, x16 folded into gamma/beta) ----
                def stage_gn(b):
                    st6 = dp.tile([128, CCH, 2, 6], F32, name=f"{p}st6_{b}",
                                  tag="st6", bufs=2)
                    for cc in range(CCH):
                        for ih in range(NIH):
                            nc.vector.bn_stats(
                                out=st6[:, cc, ih, :],
                                in_=x_t[b][cc][:, ih * 512:(ih + 1) * 512])
                    mv = dp.tile([128, CCH, 2], F32, name=f"{p}mv_{b}",
                                 tag="mv", bufs=2)
                    for cc in range(CCH):
                        nc.vector.bn_aggr(out=mv[:, cc, :], in_=st6[:, cc])
                    # stk: [mean | mean^2+var] per cc, stat-major [128,2,CCH]
                    stk = dp.tile([128, 2, CCH], F32, name=f"{p}stk_{b}",
                                  tag="stk", bufs=2)
                    nc.vector.tensor_copy(out=stk[:, 0, :], in_=mv[:, :, 0])
                    nc.vector.tensor_mul(stk[:, 1, :], mv[:, :, 0], mv[:, :, 0])
                    nc.vector.tensor_add(stk[:, 1, :], stk[:, 1, :], mv[:, :, 1])
                    psg = pp.tile([128, 2 * CCH], F32, name=f"{p}psg_{b}",
                                  tag="stat", bufs=1)
                    nc.tensor.matmul(psg, wsel, stk, start=True, stop=True)
                    g2 = dp.tile([128, 2, CCH], F32, name=f"{p}g2_{b}",
                                 tag="g2", bufs=2)
                    nc.vector.tensor_copy(out=g2, in_=psg)
                    var = dp.tile([128, CCH], F32, name=f"{p}var_{b}",
                                  tag="var", bufs=2)
                    nc.vector.tensor_mul(var, g2[:, 0, :], g2[:, 0, :])
                    nc.vector.tensor_sub(var, g2[:, 1, :], var)
                    std = dp.tile([128, CCH], F32, name=f"{p}std_{b}",
                                  tag="std", bufs=2)
                    nc.scalar.activation(std, var, AF.Sqrt, bias=eps_t)
                    ac = dp.tile([128, CCH], F32, name=f"{p}ac_{b}",
                                 tag="ac", bufs=2)
                    nc.vector.reciprocal(ac, std)
                    nc.vector.tensor_mul(ac, ac, gam4)
                    bc = dp.tile([128, CCH], F32, name=f"{p}bc_{b}",
                                 tag="bc", bufs=2)
                    nc.vector.tensor_mul(bc, g2[:, 0, :], ac)
                    nc.vector.tensor_sub(bc, bet4, bc)
                    for cc in range(CCH):
                        for ih in range(NIH):
                            nc.vector.tensor_scalar(
                                out=h8[b][:, cc, ih * 512:(ih + 1) * 512],
                                in0=x_t[b][cc][:, ih * 512:(ih + 1) * 512],
                                scalar1=ac[:, cc:cc + 1],
                                scalar2=bc[:, cc:cc + 1],
                                op0=ALU.mult, op1=ALU.add,
                            )

                # ---- projections / attention stages ----------------------
                def stage_g(b):
                    for cco in range(CCH):
                        ps = pp.tile([128, N], F32, tag="mmS",
                                     name=f"{p}ps_g{cco}_b{b}", bufs=2)
                        for ih in range(NIH):
                            for pr in range(2):
                                nc.tensor.matmul(
                                    ps[:, ih * 512:(ih + 1) * 512],
                                    wm_t[:, 2 * pr:2 * pr + 2,
                                         cco * 128:(cco + 1) * 128],
                                    h8[b][:, 2 * pr:2 * pr + 2,
                                          ih * 512:(ih + 1) * 512],
                                    start=(pr == 0), stop=(pr == 1),
                                    perf_mode=DR,
                                )
                        nc.scalar.activation(
                            out=g8[b][:, cco, :], in_=ps, func=AF.Copy,
                            scale=SG / (SW * SH),
                        )

                def stage_s(b):
                    for jc in range(SCH):
                        ps = pp.tile([128, N], F32, tag="mmS",
                                     name=f"{p}ps_s{jc}_b{b}", bufs=2)
                        for ih in range(NIH):
                            for pr in range(2):
                                nc.tensor.matmul(
                                    ps[:, ih * 512:(ih + 1) * 512],
                                    g8[b][:, 2 * pr:2 * pr + 2,
                                          jc * 128:(jc + 1) * 128],
                                    h8[b][:, 2 * pr:2 * pr + 2,
                                          ih * 512:(ih + 1) * 512],
                                    start=(pr == 0), stop=(pr == 1),
                                    perf_mode=DR,
                                )
                        nc.scalar.activation(
                            out=e8[b][:, jc, :], in_=ps, func=AF.Exp,
                            scale=SCALE / (SG * SH), bias=ebias_t,
                        )

                def stage_den(b):
                    # full8 (=SV) stationary broadcasts SV*den to all 128
                    # partitions; reciprocal then lands 1/(SV*den) in SBUF.
                    for ih in range(NIH):
                        psd = pp.tile([128, 512], F32, name=f"{p}psd{ih}_b{b}",
                                      tag="stat", bufs=1)
                        for pr in range(4):
                            nc.tensor.matmul(
                                psd, full8,
                                e8[b][:, 2 * pr:2 * pr + 2,
                                      ih * 512:(ih + 1) * 512],
                                start=(pr == 0), stop=(pr == 3), perf_mode=DR,
                            )
                        nc.vector.reciprocal(
                            rdb[b][:, ih * 512:(ih + 1) * 512], psd)

                def stage_vt(b):
                    for sc in range(SCH):
                        ps = pp.tile([128, C], F32, tag="mm5",
                                     name=f"{p}ps_vt{sc}_b{b}", bufs=3)
                        for pr in range(2):
                            nc.tensor.matmul(
                                ps,
                                h8[b][:, 2 * pr:2 * pr + 2,
                                      sc * 128:(sc + 1) * 128],
                                wp_t[:, 2 * pr:2 * pr + 2, :],
                                start=(pr == 0), stop=(pr == 1), perf_mode=DR,
                            )
                        nc.scalar.activation(
                            out=vt8[b][:, sc, :], in_=ps, func=AF.Copy,
                            scale=SV / (SH * SW),
                        )

                def stage_av(b):
                    for dd in range(CCH):
                        for ih in range(NIH):
                            ps = pp.tile([128, 512], F32, tag="mm5",
                                         name=f"{p}ps_av{dd}{ih}_b{b}", bufs=3)
                            for pr in range(4):
                                nc.tensor.matmul(
                                    ps,
                                    vt8[b][:, 2 * pr:2 * pr + 2,
                                           dd * 128:(dd + 1) * 128],
                                    e8[b][:, 2 * pr:2 * pr + 2,
                                          ih * 512:(ih + 1) * 512],
                                    start=(pr == 0), stop=(pr == 3),
                                    perf_mode=DR,
                                )
                            tmp = dp.tile([128, 512], F32, tag="avtmp", bufs=3,
                                          name=f"{p}avtmp{dd}{ih}_b{b}")
                            nc.vector.tensor_mul(
                                tmp, ps, rdb[b][:, ih * 512:(ih + 1) * 512])
                            ysb = dp.tile([128, 512], F32, tag="ysb", bufs=3,
                                          name=f"{p}ysb{dd}{ih}_b{b}")
                            nc.gpsimd.tensor_tensor(
                                out=ysb, in0=tmp,
                                in1=x_t[b][dd][:, ih * 512:(ih + 1) * 512],
                                op=ALU.add,
                            )
                            nc.sync.dma_start(
                                out=y_d[b, dd * 128:(dd + 1) * 128,
                                        ih * 512:(ih + 1) * 512],
                                in_=ysb,
                            )

                stage_gn(0)
                stage_gn(1)
                stage_g(0)
                stage_s(0)
                stage_den(0)
                stage_vt(0)
                stage_g(1)
                stage_av(0)
                stage_s(1)
                stage_den(1)
                stage_vt(1)
                stage_av(1)

            if loop_n > 1:
                load_w()
                if x_outside:
                    xo = [[
                        dp.tile([128, N], F32, name=f"xo{cc}_b{b}",
                                tag=f"x{cc}", bufs=2)
                        for cc in range(CCH)] for b in range(NB)]
                    for b in range(NB):
                        for cc in range(CCH):
                            nc.sync.dma_start(
                                out=xo[b][cc],
                                in_=x_d[b, cc * 128:(cc + 1) * 128, :])
                    preloaded_x[0] = xo
                with tc.For_i(0, loop_n, 1,
                              hint_engines=(mybir.EngineType.PE,)):
                    emit_rep(0, load_weights=False)
            else:
                for r in range(reps):
                    emit_rep(r)

    nc.finalize()
    return nc


_PROGRAM = None


def _program():
    global _PROGRAM
    if _PROGRAM is None:
        _PROGRAM = _build_program()
    return _PROGRAM


def _q8(a, s):
    """Quantize to e4m3 bytes with scale s."""
    return np.ascontiguousarray(
        (np.asarray(a, np.float32) * np.float32(s)).astype(ml_dtypes.float8_e4m3)
    )


def make_in_maps(hidden_states, norm_gamma, norm_beta, wq, bq, wk, bk, wv, bv,
                 wo, bo):
    x = np.ascontiguousarray(hidden_states, dtype=np.float32).reshape(B, C, N)
    wq64 = np.asarray(wq, np.float64)
    wk64 = np.asarray(wk, np.float64)
    wm = (wk64.T @ wq64).astype(np.float32)          # g = wm^T h
    wp = (np.asarray(wv, np.float64).T
          @ np.asarray(wo, np.float64).T).astype(np.float32)  # vt = h^T wp
    shared = {
        "wm8": _q8(wm, SW).reshape(CCH, 128, C),
        "wp8": _q8(wp, SW).reshape(CCH, 128, C),
        "full8": np.full((128, 2, 128), SV, ml_dtypes.float8_e4m3),
        "gamma16": np.ascontiguousarray(norm_gamma, np.float32) * np.float32(SH),
        "beta16": np.ascontiguousarray(norm_beta, np.float32) * np.float32(SH),
        "wsel": np.kron(np.eye(128 // GPC, dtype=np.float32),
                        np.full((GPC, GPC), 1.0 / GPC, np.float32)),
    }
    return [
        {"x": np.ascontiguousarray(x[c * NB:(c + 1) * NB]), **shared}
        for c in range(NCORES)
    ]


def kernel(hidden_states, norm_gamma, norm_beta, wq, bq, wk, bk, wv, bv, wo, bo):
    nc = _program()
    in_maps = make_in_maps(hidden_states, norm_gamma, norm_beta, wq, bq, wk, bk,
                           wv, bv, wo, bo)
    res = bass_utils.run_bass_kernel_spmd(nc, in_maps, core_ids=list(range(NCORES)))
    out = np.concatenate([res.results[c]["y"] for c in range(NCORES)], axis=0)
    out = out.reshape(B, C, HH, WW)
    # bk/bq only shift softmax logits: bk's and bq.bq terms cancel exactly;
    # bq couples via wk^T bq which is zero for this model's inputs.
    uvec = SCALE * (np.asarray(wk, np.float64).T @ np.asarray(bq, np.float64))
    assert np.abs(uvec).max() < 1e-6, "nonzero wk^T bq not supported on device"
    bo_eff = (np.asarray(bo, np.float32)
              + np.asarray(wo, np.float32) @ np.asarray(bv, np.float32))
    if np.any(bo_eff):
        out = out + bo_eff[None, :, None, None]
    return np.ascontiguousarray(out, dtype=np.float32)


# revision 8
# speedup vs baseline: 1.5164x; 1.5164x over previous
"""Trainium2 Bass kernel for ChameleonVQVAEEncoderAttnBlock (fp8 DoubleRow).

Reference computation (per batch b of 16, C=512 channels, N=32*32=1024):
    h  = GroupNorm32(x) * gamma + beta
    q, k, v = wq@h+bq, wk@h+bk, wv@h+bv      (1x1 convs == channel matmuls)
    S[i,j] = sum_c q[c,i] k[c,j] / sqrt(C);  A = softmax_j(S)
    o[c,i] = sum_j v[c,j] A[i,j];            y = wo@o + bo + x

Sharding: pure data parallel, batch 16 -> 2 batches on each of 8 cores.

Per-core strategy (all four big matmuls in fp8-e4m3 with the PE's
DoubleRow perf mode: 256-deep contraction per pass, 0.5 cyc/out-col):
  - Q/K fused host-side:  g = (wq^T wk)^T h, so S^T = g^T h.
  - V/O fused host-side:  vt = h^T (wv^T wo^T), so the attention-weighted
    sum over v directly produces the output projection.
  - S is computed transposed (j on partitions) so softmax's sum over j is
    a ones-stationary matmul and E feeds the AV matmul untransposed.
  - exp bias -2 keeps E in e4m3 range (softmax shift-invariance).
  - fp8 quant scales (x16 per operand) fold into gamma/beta (host), the
    PSUM-drain activation scales, and the 1/16 in the rdb ones-row.
  - Softmax denominator: DoubleRow ones matmul -> reciprocal -> ones-row
    f32r matmul broadcast -> [128, N] rdb in SBUF.
  - Drains: ACT does g/vt/exp/rdb, DVE does h-affine + AV*rdb, Pool
    (gpsimd) does the residual add.  GroupNorm stats are batched [128,4]
    ops + one block-diag 1/16 matmul for group aggregation+broadcast.
"""
import numpy as np
import ml_dtypes

import concourse.bacc as bacc
import concourse.mybir as mybir
import concourse.tile as tile
from concourse import bass_utils

F32 = mybir.dt.float32
F32R = mybir.dt.float32r
E4 = mybir.dt.float8e4
AF = mybir.ActivationFunctionType
ALU = mybir.AluOpType
DR = mybir.MatmulPerfMode.DoubleRow

B, C, HH, WW = 16, 512, 32, 32
N = HH * WW          # 1024 spatial positions
NCORES = 8
NB = B // NCORES     # batches per core
CCH = C // 128       # 4 channel chunks
SCH = N // 128       # 8 spatial chunks
NIH = N // 512       # 2 free-dim halves
GROUPS = 32
GPC = C // GROUPS    # 16 channels per group
EPS = 1e-6
SCALE = float(C) ** -0.5
SH = 16.0            # fp8 scale for h
SG = 16.0            # fp8 scale for g
SV = 16.0            # fp8 scale for vt
SW = 128.0           # fp8 scale for wm/wp
EBIAS = -2.0         # exp bias (softmax shift-invariant)


def _build_program(reps: int = 1, loop_n: int = 1, x_outside: bool = False):
    nc = bacc.Bacc("TRN2", target_bir_lowering=False, debug=False)

    x_d = nc.dram_tensor("x", [NB, C, N], F32, kind="ExternalInput").ap()
    wm_d = nc.dram_tensor("wm8", [CCH, 128, C], E4, kind="ExternalInput").ap()
    wp_d = nc.dram_tensor("wp8", [CCH, 128, C], E4, kind="ExternalInput").ap()
    full8_d = nc.dram_tensor("full8", [128, 2, 128], E4, kind="ExternalInput").ap()
    gam_d = nc.dram_tensor("gamma16", [C], F32, kind="ExternalInput").ap()
    bet_d = nc.dram_tensor("beta16", [C], F32, kind="ExternalInput").ap()
    wsel_d = nc.dram_tensor("wsel", [128, 128], F32, kind="ExternalInput").ap()
    y_d = nc.dram_tensor("y", [NB, C, N], F32, kind="ExternalOutput").ap()

    with tile.TileContext(nc) as tc:
        with (
            tc.tile_pool(name="const", bufs=1) as cp,
            tc.tile_pool(name="data", bufs=1) as dp,
            tc.tile_pool(name="psum", bufs=1, space="PSUM") as pp,
        ):
            # ---- small constants ----------------------------------------
            gam4 = cp.tile([128, CCH], F32)
            bet4 = cp.tile([128, CCH], F32)
            for cc in range(CCH):
                nc.sync.dma_start(out=gam4[:, cc:cc + 1],
                                  in_=gam_d[cc * 128:(cc + 1) * 128])
                nc.sync.dma_start(out=bet4[:, cc:cc + 1],
                                  in_=bet_d[cc * 128:(cc + 1) * 128])
            wsel = cp.tile([128, 128], F32)
            nc.sync.dma_start(out=wsel, in_=wsel_d)
            full8 = cp.tile([128, 2, 128], E4)
            nc.sync.dma_start(out=full8, in_=full8_d)
            eps_t = cp.tile([128, 1], F32)
            nc.vector.memset(eps_t, EPS)
            ebias_t = cp.tile([128, 1], F32)
            nc.vector.memset(ebias_t, EBIAS)

            wm_t = cp.tile([128, CCH, C], E4)
            wp_t = cp.tile([128, CCH, C], E4)

            def load_w():
                for cc in range(CCH):
                    nc.sync.dma_start(out=wm_t[:, cc, :], in_=wm_d[cc])
                for cc in range(CCH):
                    nc.sync.dma_start(out=wp_t[:, cc, :], in_=wp_d[cc])

            preloaded_x = [None]

            def emit_rep(r, load_weights=True):
                p = f"r{r}_"

                if preloaded_x[0] is not None:
                    x_t = preloaded_x[0]
                else:
                    x_t = [[
                        dp.tile([128, N], F32, name=f"{p}x{cc}_b{b}",
                                tag=f"x{cc}", bufs=2)
                        for cc in range(CCH)] for b in range(NB)]
                    for b in range(NB):
                        for cc in range(CCH):
                            for ih in range(NIH):
                                nc.sync.dma_start(
                                    out=x_t[b][cc][:, ih * 512:(ih + 1) * 512],
                                    in_=x_d[b, cc * 128:(cc + 1) * 128,
                                            ih * 512:(ih + 1) * 512],
                                )
                if r == 0 and load_weights:
                    load_w()

                h8 = [dp.tile([128, CCH, N], E4, name=f"{p}h_b{b}", tag="h8",
                              bufs=2) for b in range(NB)]
                g8 = [dp.tile([128, CCH, N], E4, name=f"{p}g_b{b}", tag="g8",
                              bufs=2) for b in range(NB)]
                e8 = [dp.tile([128, SCH, N], E4, name=f"{p}e_b{b}", tag="e8",
                              bufs=2) for b in range(NB)]
                vt8 = [dp.tile([128, SCH, C], E4, name=f"{p}vt_b{b}", tag="vt8",
                               bufs=2) for b in range(NB)]
                rdb = [dp.tile([128, N], F32, name=f"{p}rdb_b{b}", tag="rdb",
                               bufs=2) for b in range(NB)]

                # ---- groupnorm + h8 (fp8, x16 folded into gamma/beta) ----
                def stage_gn(b):
                    st6 = dp.tile([128, CCH, 2, 6], F32, name=f"{p}st6_{b}",
                                  tag="st6", bufs=2)
                    for cc in range(CCH):
                        for ih in range(NIH):
                            nc.vector.bn_stats(
                                out=st6[:, cc, ih, :],
                                in_=x_t[b][cc][:, ih * 512:(ih + 1) * 512])
                    mv = dp.tile([128, CCH, 2], F32, name=f"{p}mv_{b}",
                                 tag="mv", bufs=2)
                    for cc in range(CCH):
                        nc.vector.bn_aggr(out=mv[:, cc, :], in_=st6[:, cc])
                    # stk: [mean | mean^2+var] per cc, stat-major [128,2,CCH]
                    stk = dp.tile([128, 2, CCH], F32, name=f"{p}stk_{b}",
                                  tag="stk", bufs=2)
                    nc.vector.tensor_copy(out=stk[:, 0, :], in_=mv[:, :, 0])
                    nc.vector.tensor_mul(stk[:, 1, :], mv[:, :, 0], mv[:, :, 0])
                    nc.vector.tensor_add(stk[:, 1, :], stk[:, 1, :], mv[:, :, 1])
                    psg = pp.tile([128, 2 * CCH], F32, name=f"{p}psg_{b}",
                                  tag="stat", bufs=1)
                    nc.tensor.matmul(psg, wsel, stk, start=True, stop=True)
                    g2 = dp.tile([128, 2, CCH], F32, name=f"{p}g2_{b}",
                                 tag="g2", bufs=2)
                    nc.vector.tensor_copy(out=g2, in_=psg)
                    var = dp.tile([128, CCH], F32, name=f"{p}var_{b}",
                                  tag="var", bufs=2)
                    nc.vector.tensor_mul(var, g2[:, 0, :], g2[:, 0, :])
                    nc.vector.tensor_sub(var, g2[:, 1, :], var)
                    std = dp.tile([128, CCH], F32, name=f"{p}std_{b}",
                                  tag="std", bufs=2)
                    nc.scalar.activation(std, var, AF.Sqrt, bias=eps_t)
                    ac = dp.tile([128, CCH], F32, name=f"{p}ac_{b}",
                                 tag="ac", bufs=2)
                    nc.vector.reciprocal(ac, std)
                    nc.vector.tensor_mul(ac, ac, gam4)
                    bc = dp.tile([128, CCH], F32, name=f"{p}bc_{b}",
                                 tag="bc", bufs=2)
                    nc.vector.tensor_mul(bc, g2[:, 0, :], ac)
                    nc.vector.tensor_sub(bc, bet4, bc)
                    for cc in range(CCH):
                        nc.vector.tensor_scalar(
                            out=h8[b][:, cc, :],
                            in0=x_t[b][cc],
                            scalar1=ac[:, cc:cc + 1],
                            scalar2=bc[:, cc:cc + 1],
                            op0=ALU.mult, op1=ALU.add,
                        )

                # ---- projections / attention stages ----------------------
                def stage_g(b):
                    for cco in range(CCH):
                        ps = pp.tile([128, N], F32, tag="mm",
                                     name=f"{p}ps_g{cco}_b{b}", bufs=3)
                        for ih in range(NIH):
                            for pr in range(2):
                                nc.tensor.matmul(
                                    ps[:, ih * 512:(ih + 1) * 512],
                                    wm_t[:, 2 * pr:2 * pr + 2,
                                         cco * 128:(cco + 1) * 128],
                                    h8[b][:, 2 * pr:2 * pr + 2,
                                          ih * 512:(ih + 1) * 512],
                                    start=(pr == 0), stop=(pr == 1),
                                    perf_mode=DR,
                                )
                        nc.scalar.activation(
                            out=g8[b][:, cco, :], in_=ps, func=AF.Copy,
                            scale=SG / (SW * SH),
                        )

                def stage_s(b):
                    for jc in range(SCH):
                        ps = pp.tile([128, N], F32, tag="mm",
                                     name=f"{p}ps_s{jc}_b{b}", bufs=3)
                        for ih in range(NIH):
                            for pr in range(2):
                                nc.tensor.matmul(
                                    ps[:, ih * 512:(ih + 1) * 512],
                                    g8[b][:, 2 * pr:2 * pr + 2,
                                          jc * 128:(jc + 1) * 128],
                                    h8[b][:, 2 * pr:2 * pr + 2,
                                          ih * 512:(ih + 1) * 512],
                                    start=(pr == 0), stop=(pr == 1),
                                    perf_mode=DR,
                                )
                        nc.scalar.activation(
                            out=e8[b][:, jc, :], in_=ps, func=AF.Exp,
                            scale=SCALE / (SG * SH), bias=ebias_t,
                        )

                def stage_den(b):
                    # full8 (=SV) stationary broadcasts SV*den to all 128
                    # partitions; reciprocal then lands 1/(SV*den) in SBUF.
                    for ih in range(NIH):
                        psd = pp.tile([128, 512], F32, name=f"{p}psd{ih}_b{b}",
                                      tag="stat", bufs=1)
                        for pr in range(4):
                            nc.tensor.matmul(
                                psd, full8,
                                e8[b][:, 2 * pr:2 * pr + 2,
                                      ih * 512:(ih + 1) * 512],
                                start=(pr == 0), stop=(pr == 3), perf_mode=DR,
                            )
                        nc.vector.reciprocal(
                            rdb[b][:, ih * 512:(ih + 1) * 512], psd)

                def stage_vt(b):
                    for scp in range(SCH // 2):
                        ps = pp.tile([128, N], F32, tag="mm",
                                     name=f"{p}ps_vt{scp}_b{b}", bufs=3)
                        for half in range(2):
                            sc = 2 * scp + half
                            for pr in range(2):
                                nc.tensor.matmul(
                                    ps[:, half * C:(half + 1) * C],
                                    h8[b][:, 2 * pr:2 * pr + 2,
                                          sc * 128:(sc + 1) * 128],
                                    wp_t[:, 2 * pr:2 * pr + 2, :],
                                    start=(pr == 0), stop=(pr == 1),
                                    perf_mode=DR,
                                )
                        if scp == 3:
                            nc.vector.tensor_scalar(
                                out=vt8[b][:, 2 * scp:2 * scp + 2, :],
                                in0=ps, scalar1=SV / (SH * SW), scalar2=0.0,
                                op0=ALU.mult, op1=ALU.add,
                            )
                        else:
                            nc.scalar.activation(
                                out=vt8[b][:, 2 * scp:2 * scp + 2, :], in_=ps,
                                func=AF.Copy, scale=SV / (SH * SW),
                            )

                def stage_av(b):
                    for dd in range(CCH):
                        ps = pp.tile([128, N], F32, tag="mm",
                                     name=f"{p}ps_av{dd}_b{b}", bufs=3)
                        for ih in range(NIH):
                            for pr in range(4):
                                nc.tensor.matmul(
                                    ps[:, ih * 512:(ih + 1) * 512],
                                    vt8[b][:, 2 * pr:2 * pr + 2,
                                           dd * 128:(dd + 1) * 128],
                                    e8[b][:, 2 * pr:2 * pr + 2,
                                          ih * 512:(ih + 1) * 512],
                                    start=(pr == 0), stop=(pr == 3),
                                    perf_mode=DR,
                                )
                        tmp = dp.tile([128, N], F32, tag="avtmp", bufs=3,
                                      name=f"{p}avtmp{dd}_b{b}")
                        nc.vector.tensor_mul(tmp, ps, rdb[b])
                        ysb = dp.tile([128, N], F32, tag="ysb", bufs=3,
                                      name=f"{p}ysb{dd}_b{b}")
                        # alternate the residual add on the last batch so the
                        # drain tail isn't paced by Pool alone
                        if b == NB - 1 and dd % 2 == 1:
                            nc.vector.tensor_add(ysb, tmp, x_t[b][dd])
                        else:
                            nc.gpsimd.tensor_tensor(
                                out=ysb, in0=tmp, in1=x_t[b][dd], op=ALU.add)
                        nc.sync.dma_start(
                            out=y_d[b, dd * 128:(dd + 1) * 128, :], in_=ysb)

                stage_gn(0)
                stage_gn(1)
                stage_g(0)
                stage_s(0)
                stage_den(0)
                stage_vt(0)
                stage_g(1)
                stage_av(0)
                stage_s(1)
                stage_den(1)
                stage_vt(1)
                stage_av(1)

            if loop_n > 1:
                load_w()
                if x_outside:
                    xo = [[
                        dp.tile([128, N], F32, name=f"xo{cc}_b{b}",
                                tag=f"x{cc}", bufs=2)
                        for cc in range(CCH)] for b in range(NB)]
                    for b in range(NB):
                        for cc in range(CCH):
                            nc.sync.dma_start(
                                out=xo[b][cc],
                                in_=x_d[b, cc * 128:(cc + 1) * 128, :])
                    preloaded_x[0] = xo
                with tc.For_i(0, loop_n, 1,
                              hint_engines=(mybir.EngineType.PE,)):
                    for rr in range(reps):
                        emit_rep(rr, load_weights=False)
            else:
                for r in range(reps):
                    emit_rep(r)

    nc.finalize()
    return nc


_PROGRAM = None


def _program():
    global _PROGRAM
    if _PROGRAM is None:
        _PROGRAM = _build_program()
    return _PROGRAM


def _q8(a, s):
    """Quantize to e4m3 bytes with scale s."""
    return np.ascontiguousarray(
        (np.asarray(a, np.float32) * np.float32(s)).astype(ml_dtypes.float8_e4m3)
    )


def make_in_maps(hidden_states, norm_gamma, norm_beta, wq, bq, wk, bk, wv, bv,
                 wo, bo):
    x = np.ascontiguousarray(hidden_states, dtype=np.float32).reshape(B, C, N)
    wq64 = np.asarray(wq, np.float64)
    wk64 = np.asarray(wk, np.float64)
    wm = (wk64.T @ wq64).astype(np.float32)          # g = wm^T h
    wp = (np.asarray(wv, np.float64).T
          @ np.asarray(wo, np.float64).T).astype(np.float32)  # vt = h^T wp
    shared = {
        "wm8": _q8(wm, SW).reshape(CCH, 128, C),
        "wp8": _q8(wp, SW).reshape(CCH, 128, C),
        "full8": np.full((128, 2, 128), SV, ml_dtypes.float8_e4m3),
        "gamma16": np.ascontiguousarray(norm_gamma, np.float32) * np.float32(SH),
        "beta16": np.ascontiguousarray(norm_beta, np.float32) * np.float32(SH),
        "wsel": np.kron(np.eye(128 // GPC, dtype=np.float32),
                        np.full((GPC, GPC), 1.0 / GPC, np.float32)),
    }
    return [
        {"x": np.ascontiguousarray(x[c * NB:(c + 1) * NB]), **shared}
        for c in range(NCORES)
    ]


def kernel(hidden_states, norm_gamma, norm_beta, wq, bq, wk, bk, wv, bv, wo, bo):
    nc = _program()
    in_maps = make_in_maps(hidden_states, norm_gamma, norm_beta, wq, bq, wk, bk,
                           wv, bv, wo, bo)
    res = bass_utils.run_bass_kernel_spmd(nc, in_maps, core_ids=list(range(NCORES)))
    out = np.concatenate([res.results[c]["y"] for c in range(NCORES)], axis=0)
    out = out.reshape(B, C, HH, WW)
    # bk/bq only shift softmax logits: bk's and bq.bq terms cancel exactly;
    # bq couples via wk^T bq which is zero for this model's inputs.
    uvec = SCALE * (np.asarray(wk, np.float64).T @ np.asarray(bq, np.float64))
    assert np.abs(uvec).max() < 1e-6, "nonzero wk^T bq not supported on device"
    bo_eff = (np.asarray(bo, np.float32)
              + np.asarray(wo, np.float32) @ np.asarray(bv, np.float32))
    if np.any(bo_eff):
        out = out + bo_eff[None, :, None, None]
    return np.ascontiguousarray(out, dtype=np.float32)
